# revision 7
# baseline (speedup 1.0000x reference)
"""Trainium2 Bass kernel for the BillehColumn GLIF3 spiking network.

Strategy
--------
Batch-parallel: each of the 8 NeuronCores simulates one batch element
end-to-end with all state resident in SBUF; no inter-core communication.

The sparse input projection (seg_mm over in_src/in_tgt/w_in with the binary
spike raster x) is turned into dense per-step "weight images": the host lays
out each step's active input-edge weights at their target positions in the
[128, 1564] accumulator layout (pure layout/selection, no arithmetic);
duplicate targets go to extra layers.  The device sums the K layers plus the
background image plus the decayed psc_rise state with identity matmuls on the
PE into PSUM, so the vector engines never touch the image summation.

State recurrences are algebraically rescaled so only three NR-sized tensors
evolve per step (all fp16, DVE 2x mode):

    n_t   = sd * w_{t-1}              (DVE)
    w_t   = n_t + bkg + sum_k img_k   (PE accumulate, ACT copies PSUM->SBUF)
    m_t   = cfpi * n_t                (DVE)   cfpi = current_factor*psc_initial
    p_t   = sd * p_{t-1} + m_t        (DVE x2)
    ic_t  = sum_r p_{t-1}             (Pool reduce, fp32)
    y_t   = decay * y_{t-1} + ic_t + c2   (Pool add + DVE x2, fp32)
    z_t   = y_t > 0                   (DVE tensor_scalar, fp16 out)

with w = raw psc_rise integrator (psc_rise = psc_initial * w), p = cf-scaled
psc, y = v - v_th, and c2 = decay*v_th - v_th + cf*param_g*e_l.  The
asc/refractory/reset terms all carry a prev-spike factor and are identically
zero while no spike occurs before the final step; the host verifies that on
the device output and falls back to an exact numpy recompute otherwise.
"""

import numpy as np

import concourse.bass as bass
import concourse.mybir as mybir
import concourse.tile as tile
from concourse.bass_utils import run_bass_kernel_spmd
from concourse.masks import make_identity

from concourse.vector_clock import ScopedClock

# ---- inlined walrus workarounds (sync-wait splitting) ----

MAX_WAITS = 1


def _split_drain_and_barrier(self, tick_clock, wait_clock):
    drain_inst = self.nc.sync.drain()
    wait_clock.add_sem_waits(
        drain_inst.ins, ScopedClock({None: tick_clock.global_clock})
    )
    si = drain_inst.ins.sync_info
    if si is not None and si.on_wait and len(si.on_wait) > MAX_WAITS:
        waits = list(si.on_wait)
        si.on_wait = waits[:MAX_WAITS]
        rest = waits[MAX_WAITS:]
        for i in range(0, len(rest), MAX_WAITS):
            extra = self.nc.sync.drain()
            esi = extra.ins.sync_info
            if esi is None:
                extra.ins.sync_info = mybir.SyncInfo(
                    on_wait=rest[i : i + MAX_WAITS], on_update=[]
                )
            else:
                esi.on_wait = rest[i : i + MAX_WAITS]

    self.nc.all_engine_barrier()
    assert self.sems is not None
    popped = self.nc._tile_sem_poison_stack.pop()
    assert popped is self._sem_poison
    _clear_sems_chunked(self.nc, list(self.sems.allocated().values()))
    self.nc.all_engine_barrier()


def _clear_sems_chunked(nc, sems, max_range=3):
    """clear_and_free_semaphores, but with EVENT_SEMAPHORE_RANGE_CLEAR ranges
    capped at max_range sems — longer ranges hit "ISA wrong length" in this
    walrus build."""
    if not sems:
        return
    sem_nums = sorted(
        s.num if not isinstance(s, int) else s for s in sems
    )
    runs = []
    start = prev = sem_nums[0]
    for n in sem_nums[1:]:
        if n == prev + 1:
            prev = n
            continue
        runs.append((start, prev))
        start = prev = n
    runs.append((start, prev))
    for a, b in runs:
        lo = a
        while lo <= b:
            hi = min(lo + max_range - 1, b)
            r = range(lo, hi + 1)
            assert nc._state.free_isdisjoint(r)
            nc.gpsimd.dma_reset(r)
            nc.gpsimd.sem_clear(r)
            lo = hi + 1
    nc._state.prepend_free_semaphores(sem_nums)
    for poison_set in nc._tile_sem_poison_stack:
        poison_set.update(sem_nums)


tile.TileContext._drain_and_barrier = _split_drain_and_barrier


def split_excess_waits(nc, max_waits: int = MAX_WAITS):
    """Move excess sem waits onto same-engine nops inserted before the
    instruction. Call after the TileContext has exited, before compiling."""
    n_split = 0
    for fn in nc.m.functions:
        for bb in fn.blocks:
            out = []
            for inst in bb.instructions:
                si = inst.sync_info
                if si is not None and si.on_wait and len(si.on_wait) > max_waits:
                    waits = list(si.on_wait)
                    rest, keep = waits[:-max_waits], waits[-max_waits:]
                    for i in range(0, len(rest), max_waits):
                        nop = mybir.InstNoOp(
                            name=f"{inst.name}-wsplit{i}",
                            engine=inst.engine,
                            bass_nofuse=True,
                            sync_info=mybir.SyncInfo(
                                on_wait=rest[i : i + max_waits], on_update=[]
                            ),
                        )
                        out.append(nop)
                    si.on_wait = keep
                    n_split += 1
                out.append(inst)
            _replace_instructions(bb, out)
    return n_split


def _replace_instructions(bb, insts):
    try:
        bb.instructions = insts
        return
    except Exception:
        pass
    cur = bb.instructions
    if isinstance(cur, list):
        cur.clear()
        cur.extend(insts)
        return
    raise RuntimeError(f"cannot replace instructions on {type(bb)}")

# ---- end inlined workarounds ----


F32 = mybir.dt.float32
F16 = mybir.dt.float16
Alu = mybir.AluOpType
AF = mybir.ActivationFunctionType

N = 50000
R = 4
B = 8
T = 10
N_IN = 17400
P = 128
CW = 391            # columns for N-sized state: 128*391 = 50048 >= N
NP = P * CW
NRW = CW * R        # 1564 columns for (n, r) state
CHUNK = 512         # PSUM bank: 512 fp32 columns per matmul region

_cache = {}


def _layout_n(a):
    """[N] -> [128, 391] (pad 0)."""
    out = np.zeros((NP,), np.float32)
    out[:N] = a
    return out.reshape(P, CW)


def _layout_nr(a):
    """[N, R] -> [128, 1564] r-major: col = r * CW + (n % CW)."""
    out = np.zeros((NP, R), np.float32)
    out[:N] = a
    return (
        out.reshape(P, CW, R).transpose(0, 2, 1).reshape(P, R * CW)
    )


def _acc_col(rn):
    n = rn // R
    r = rn % R
    return n // CW, r * CW + (n % CW)


def _build_images(x_b, in_src, in_tgt, w_in):
    """Per-step layered weight images for one batch element (fp16).

    Returns (imgs [T, K, P, NRW] f16, ximgs or None, K). Host work is
    selection + layout (+ dtype cast) only; all arithmetic involving the
    weights happens on device.
    """
    order = np.argsort(in_src, kind="stable")
    src_s = in_src[order]
    tgt_s = in_tgt[order]
    w_s = w_in[order].astype(np.float16)
    starts = np.searchsorted(src_s, np.arange(N_IN))
    ends = np.searchsorted(src_s, np.arange(N_IN) + 1)

    x_bin = bool(np.all((x_b == 0) | (x_b == 1)))
    p_all, c_all = _acc_col(tgt_s)

    per_t = []
    K = 1
    for t in range(T):
        act = np.nonzero(x_b[t])[0]
        segs = [np.arange(starts[i], ends[i]) for i in act]
        xvals = [np.full(ends[i] - starts[i], x_b[t, i], np.float16) for i in act]
        if segs:
            e = np.concatenate(segs)
            xv = np.concatenate(xvals)
        else:
            e = np.zeros((0,), np.int64)
            xv = np.zeros((0,), np.float16)
        flat = p_all[e].astype(np.int64) * NRW + c_all[e]
        order2 = np.argsort(flat, kind="stable")
        flat_s = flat[order2]
        uniq, inv, cnt = np.unique(flat_s, return_inverse=True, return_counts=True)
        first_pos = np.concatenate(([0], np.cumsum(cnt)[:-1]))
        layer = np.arange(len(flat_s)) - first_pos[inv]
        K = max(K, int(layer.max()) + 1 if len(layer) else 1)
        per_t.append((e[order2], flat_s, layer, xv[order2]))

    imgs = np.zeros((T, K, P, NRW), np.float16)
    for t, (e, flat_s, layer, xv) in enumerate(per_t):
        w_e = w_s[e] if len(e) else np.zeros((0,), np.float16)
        img = imgs[t].reshape(K, P * NRW)
        img[layer, flat_s] = w_e
    if not x_bin:
        ximgs = np.zeros((T, K, P, NRW), np.float16)
        for t, (e, flat_s, layer, xv) in enumerate(per_t):
            xi = ximgs[t].reshape(K, P * NRW)
            xi[layer, flat_s] = xv
        return imgs, ximgs, K
    return imgs, None, K


def _build_program(K, x_bin):
    nc = bass.Bass()

    def par_n(name):
        return nc.declare_dram_parameter(name, [P, CW], F32, isOutput=False)

    d_wimg = nc.declare_dram_parameter("wimg", [T * K, P, NRW], F16, isOutput=False)
    d_ximg = (
        nc.declare_dram_parameter("ximg", [T * K, P, NRW], F16, isOutput=False)
        if not x_bin
        else None
    )
    d_sd16 = nc.declare_dram_parameter("sd16", [P, NRW], F16, isOutput=False)
    d_bkg16 = nc.declare_dram_parameter("bkg16", [P, NRW], F16, isOutput=False)
    d_pi = nc.declare_dram_parameter("pi", [P, NRW], F32, isOutput=False)
    d_cfr = nc.declare_dram_parameter("cfr", [P, NRW], F32, isOutput=False)
    d_decay = par_n("decay")
    d_vth = par_n("vth")
    d_cf = par_n("cf")
    d_pg = par_n("pg")
    d_el = par_n("el")
    d_v0 = par_n("v0")
    d_z = nc.declare_dram_parameter("z", [T, P, CW], F16, isOutput=True)

    chunks = []
    lo = 0
    while lo < NRW:
        chunks.append((lo, min(NRW, lo + CHUNK)))
        lo += CHUNK

    with tile.TileContext(nc) as tc:
        with (
            tc.tile_pool(name="state", bufs=1) as st,
            tc.tile_pool(name="io", bufs=2) as io,
            tc.tile_pool(name="psum", bufs=2, space="PSUM") as pp,
        ):
            def load(dram, shape, dt):
                t_ = st.tile(shape, dt, tag=dram.name)
                nc.sync.dma_start(out=t_[:], in_=dram[:])
                return t_

            sd16 = load(d_sd16, [P, NRW], F16)
            bkg16 = load(d_bkg16, [P, NRW], F16)
            pi = load(d_pi, [P, NRW], F32)
            cfr = load(d_cfr, [P, NRW], F32)
            decay = load(d_decay, [P, CW], F32)
            vth = load(d_vth, [P, CW], F32)
            cf = load(d_cf, [P, CW], F32)
            pg = load(d_pg, [P, CW], F32)
            el = load(d_el, [P, CW], F32)
            v0 = load(d_v0, [P, CW], F32)

            # ---- derived constants ----
            # cfpi16 = fp16(current_factor * psc_initial)   [NR]
            cfpi32 = st.tile([P, NRW], F32)
            nc.vector.tensor_mul(out=cfpi32[:], in0=cfr[:], in1=pi[:])
            cfpi16 = st.tile([P, NRW], F16)
            nc.vector.tensor_copy(out=cfpi16[:], in_=cfpi32[:])
            # c2 = decay*vth - vth + cf*pg*el   [CW]
            gel = st.tile([P, CW], F32)
            nc.vector.tensor_mul(out=gel[:], in0=pg[:], in1=el[:])
            nc.vector.tensor_mul(out=gel[:], in0=cf[:], in1=gel[:])
            c2f = st.tile([P, CW], F32)
            nc.gpsimd.tensor_mul(out=c2f[:], in0=decay[:], in1=vth[:])
            nc.gpsimd.tensor_sub(out=c2f[:], in0=c2f[:], in1=vth[:])
            nc.vector.tensor_add(out=c2f[:], in0=c2f[:], in1=gel[:])
            c2 = st.tile([P, CW], F16)
            nc.vector.tensor_copy(out=c2[:], in_=c2f[:])
            decay16 = st.tile([P, CW], F16)
            nc.vector.tensor_copy(out=decay16[:], in_=decay[:])
            # y = v0 - vth  (fp16)
            yf = st.tile([P, CW], F32)
            nc.gpsimd.tensor_sub(out=yf[:], in0=v0[:], in1=vth[:])
            y = st.tile([P, CW], F16)
            nc.vector.tensor_copy(out=y[:], in_=yf[:])

            ident = st.tile([P, P], F16)
            make_identity(nc, ident[:])

            # ---- state (ping-pong) ----
            wb = [st.tile([P, NRW], F16, tag=f"wb{i}", name=f"wb{i}") for i in range(2)]
            pb = [st.tile([P, NRW], F16, tag=f"pb{i}", name=f"pb{i}") for i in range(2)]
            nc.vector.memset(wb[0][:], 0.0)
            nc.vector.memset(pb[0][:], 0.0)

            n16 = [st.tile([P, NRW], F16, tag=f"n16{i}", name=f"n16{i}") for i in range(2)]
            mh = st.tile([P, NRW], F16)
            qh = st.tile([P, NRW], F16)
            icA = st.tile([P, CW], F16)
            icB = st.tile([P, CW], F16)
            ic = st.tile([P, CW], F16)
            wsum = [st.tile([P, CW], F16, tag=f"ws{i}", name=f"ws{i}") for i in range(2)]
            y1 = st.tile([P, CW], F16)
            z16 = [st.tile([P, CW], F16, tag=f"z{i}", name=f"z{i}") for i in range(2)]

            # ---------------- time loop ----------------
            for t in range(T):
                cur, nxt = t % 2, (t + 1) % 2
                imgs = []
                for k in range(K):
                    w_ = io.tile([P, NRW], F16, tag=f"wimg{k}")
                    nc.sync.dma_start(out=w_[:], in_=d_wimg[t * K + k])
                    imgs.append(w_)
                if not x_bin:
                    for k in range(K):
                        x_ = io.tile([P, NRW], F16, tag=f"ximg{k}")
                        nc.sync.dma_start(out=x_[:], in_=d_ximg[t * K + k])
                        nc.vector.tensor_mul(out=imgs[k][:], in0=imgs[k][:],
                                             in1=x_[:])

                # n_t = sd * w_{t-1}   (fp16, DVE)
                nc.vector.tensor_mul(out=n16[cur][:], in0=sd16[:], in1=wb[cur][:])

                # PE: ps = bkg + sum_k img_k + n_t
                ps = pp.tile([P, NRW], F32, space="PSUM", tag="ps")
                layers = [bkg16] + imgs + [n16[cur]]
                for li, lay in enumerate(layers):
                    for (lo_, hi_) in chunks:
                        nc.tensor.matmul(
                            out=ps[:, lo_:hi_], lhsT=ident[:],
                            rhs=lay[:, lo_:hi_], start=(li == 0),
                            stop=(li == len(layers) - 1), skip_group_check=True,
                        )
                # ACT: w_t = fp16(ps)
                nc.scalar.copy(out=wb[nxt][:], in_=ps[:])

                # Pool: ic_t = sum_r p_{t-1} ; wsum = ic + c2   (r-major slices)
                pcur = pb[cur]
                nc.gpsimd.tensor_add(out=icA[:], in0=pcur[:, 0:CW],
                                     in1=pcur[:, CW:2 * CW])
                nc.gpsimd.tensor_add(out=icB[:], in0=pcur[:, 2 * CW:3 * CW],
                                     in1=pcur[:, 3 * CW:4 * CW])
                nc.gpsimd.tensor_add(out=ic[:], in0=icA[:], in1=icB[:])
                nc.gpsimd.tensor_add(out=wsum[cur][:], in0=ic[:], in1=c2[:])

                # DVE: p_t = sd * p_{t-1} + cfpi * n_t
                nc.vector.tensor_mul(out=mh[:], in0=cfpi16[:], in1=n16[cur][:])
                nc.vector.tensor_mul(out=qh[:], in0=sd16[:], in1=pb[cur][:])
                nc.vector.tensor_add(out=pb[nxt][:], in0=qh[:], in1=mh[:])

                # DVE: y = decay*y + wsum ; z = y > 0
                nc.vector.tensor_mul(out=y1[:], in0=decay16[:], in1=y[:])
                nc.vector.tensor_add(out=y[:], in0=y1[:], in1=wsum[cur][:])
                nc.vector.tensor_scalar(out=z16[cur][:], in0=y[:], scalar1=0.0,
                                        scalar2=None, op0=Alu.is_gt)
                nc.sync.dma_start(out=d_z[t], in_=z16[cur][:])

    split_excess_waits(nc)
    return nc


def _prep_inputs(inputs):
    x = np.asarray(inputs["x"], np.float32)
    in_src = np.asarray(inputs["in_src"])
    in_tgt = np.asarray(inputs["in_tgt"])
    w_in = np.asarray(inputs["w_in"], np.float32)

    K_all = 1
    built = []
    for b in range(B):
        imgs, ximgs, K = _build_images(x[:, b], in_src, in_tgt, w_in)
        built.append((imgs, ximgs))
        K_all = max(K_all, K)
    x_bin = all(x2 is None for _, x2 in built)

    bkg_img = np.zeros((P, NRW), np.float16)
    p_b, c_b = _acc_col(np.arange(R * N))
    bkg_img[p_b, c_b] = np.asarray(inputs["bkg_w"], np.float32).astype(np.float16)

    cf = np.asarray(inputs["current_factor"], np.float32)
    base = dict(
        sd16=_layout_nr(np.asarray(inputs["syn_decay"], np.float32)).astype(np.float16),
        bkg16=bkg_img,
        pi=_layout_nr(np.asarray(inputs["psc_initial"], np.float32)),
        cfr=_layout_nr(np.repeat(cf[:, None], R, axis=1)),
        decay=_layout_n(np.asarray(inputs["decay"], np.float32)),
        vth=_layout_n(np.asarray(inputs["v_th"], np.float32)),
        cf=_layout_n(cf),
        pg=_layout_n(np.asarray(inputs["param_g"], np.float32)),
        el=_layout_n(np.asarray(inputs["e_l"], np.float32)),
    )

    v0 = np.asarray(inputs["v0"], np.float32)
    in_maps = []
    for b in range(B):
        imgs, ximgs = built[b]
        Kb = imgs.shape[1]
        wimg = np.zeros((T, K_all, P, NRW), np.float16)
        wimg[:, :Kb] = imgs
        m = dict(base)
        m["wimg"] = wimg.reshape(T * K_all, P, NRW)
        if not x_bin:
            xim = np.zeros((T, K_all, P, NRW), np.float16)
            if ximgs is not None:
                xim[:, :Kb] = ximgs
            m["ximg"] = xim.reshape(T * K_all, P, NRW)
        m["v0"] = _layout_n(v0[b])
        in_maps.append(m)
    return in_maps, K_all, x_bin


def _reference_numpy(inputs):
    """Full-precision host recompute; used only when the device result shows
    spikes before the final step (then asc/refractory/recurrent terms
    matter) or the normalized-threshold rewrite is invalid."""
    f = np.float32
    D = 5
    x = np.asarray(inputs["x"], f)
    w_rec = np.asarray(inputs["w_rec"], f)
    rec_src = np.asarray(inputs["rec_src"])
    rec_tgt = np.asarray(inputs["rec_tgt"])
    w_in = np.asarray(inputs["w_in"], f)
    in_src = np.asarray(inputs["in_src"])
    in_tgt = np.asarray(inputs["in_tgt"])
    bkg_w = np.asarray(inputs["bkg_w"], f)
    decay = np.asarray(inputs["decay"], f)
    cf = np.asarray(inputs["current_factor"], f)
    v_th = np.asarray(inputs["v_th"], f)
    e_l = np.asarray(inputs["e_l"], f)
    v_reset = np.asarray(inputs["v_reset"], f)
    t_ref = np.asarray(inputs["t_ref"], f)
    asc_amps = np.asarray(inputs["asc_amps"], f)
    param_k = np.asarray(inputs["param_k"], f)
    param_g = np.asarray(inputs["param_g"], f)
    sd = np.asarray(inputs["syn_decay"], f)
    pi_ = np.asarray(inputs["psc_initial"], f)
    v = np.asarray(inputs["v0"], f).copy()

    k = 1.0 / (1.0 + np.exp(-param_k, dtype=f))
    asc_decay = np.exp(-k, dtype=f)
    z_buf = np.zeros((B, D * N), f)
    r = np.zeros((B, N), f)
    a1 = np.zeros((B, N), f)
    a2 = np.zeros((B, N), f)
    psc_rise = np.zeros((B, N, R), f)
    psc = np.zeros((B, N, R), f)
    zs = np.zeros((T, B, N), f)
    for t in range(T):
        prev_z = z_buf[:, :N]
        tot = np.zeros((B, R * N), f)
        act = z_buf[:, rec_src]            # [B, E]
        np.add.at(tot, (slice(None), rec_tgt), w_rec[None] * act)
        actx = x[t][:, in_src]
        np.add.at(tot, (slice(None), in_tgt), w_in[None] * actx)
        tot += bkg_w[None]
        tot = tot.reshape(B, N, R)
        new_pr = sd * psc_rise + pi_ * tot
        new_p = psc * sd + sd * psc_rise
        new_r = np.maximum(r + prev_z * t_ref - 1.0, 0.0)
        a1 = asc_decay[:, 0] * a1 + prev_z * asc_amps[:, 0]
        a2 = asc_decay[:, 1] * a2 + prev_z * asc_amps[:, 1]
        ic = psc.sum(-1, dtype=f)  # reference uses the pre-update psc
        c1 = ic + a1 + a2 + param_g * e_l
        v = decay * v + cf * c1 + prev_z * (v_reset - v_th)
        z = ((v - v_th) / (v_th - e_l) > 0.0).astype(f)
        z = np.where(new_r > 0.0, f(0.0), z)
        zs[t] = z
        z_buf = np.concatenate([z, z_buf[:, :-N]], axis=1)
        psc_rise, psc, r = new_pr, new_p, new_r
    return zs


def kernel(**inputs):
    vth = np.asarray(inputs["v_th"], np.float32)
    el = np.asarray(inputs["e_l"], np.float32)
    if not np.all(vth - el > 0):
        # (v - v_th)/(v_th - e_l) > 0 is not equivalent to v > v_th
        return _reference_numpy(inputs)

    in_maps, K, x_bin = _prep_inputs(inputs)
    key = (K, x_bin)
    if key not in _cache:
        _cache[key] = _build_program(K, x_bin)
    nc = _cache[key]
    res = run_bass_kernel_spmd(nc, in_maps, list(range(B)))
    out = np.zeros((T, B, N), np.float32)
    for b in range(B):
        z = np.asarray(res.results[b]["z"], np.float32).reshape(T, NP)
        out[:, b, :] = z[:, :N]
    if out[: T - 1].any():
        # spikes before the last step: asc/refractory/reset/recurrent terms
        # (all dropped on device) become active -> exact host recompute.
        return _reference_numpy(inputs)
    return out
